# revision 5
# baseline (speedup 1.0000x reference)
"""Trainium2 Bass kernel: LSTM neighbor-sequence aggregator + projection.

Model (reference): for each node v, run an LSTM (H=256) over the features
(F=128) of the targets of v's outgoing edges (in original edge order), take
the hidden state at the last valid step, concat with v's own features, and
project with W_out ([F+H, OUT]).

Strategy (v2)
-------------
Host (numpy):
  * Edges sorted by src (stable) -> per-node neighbor id lists.
  * Nodes dealt round-robin by global degree rank onto 8 cores; a shared
    non-increasing step schedule M_t places each node at a column whose
    lifetime equals its degree (leftover columns are dummies).
  * Gate rows are reordered [i, f, o, g] so sigmoid blocks are contiguous.
  * Wide-phase neighbor features are packed as fp8e4 DoubleRow pairs
    [128, 2, S_wide]: slot0 = e4m3(x), slot1 = e4m3(x - slot0) (residual),
    except slot1 row0 = 1.0 which carries the bias via the weight tensor.

Device (Bass/Tile, identical program on 8 cores):
  * Wide steps (M_t > 512): per 512-column chunk, 16 fp8 DoubleRow matmuls
    (8 gate blocks x {x-pairs incl. bias channel, h0/h1 pairs}) accumulate
    gates in two 4-bank PSUM tiles (ping-pong).  3 grouped ACTs (sigmoid
    over i,f; sigmoid over o; tanh over g) emit G in bf16 -- the bias rides
    the x matmul, so no per-block ACT bias is needed.  tanh(c) runs on the
    vector engine as a 2-pass custom-DVE deg-4 odd polynomial (c stays in
    [-2.5, 2.5]; inputs clamped at 2.75 in-op).  GPSIMD computes i*g.
    h is stored as fp8 DoubleRow pairs [128, 2, CH]; a bf16 copy of each
    column's FINAL h is written only for the narrow slice of columns whose
    lifetime ends at each step (projection quality).
  * Tail steps (M_t <= 512) reuse the proven bf16 machinery: bf16 weights
    and x stream, per-block or packed-bank matmuls, bias via ACT (mid-tail)
    or delta-pattern matmul (deep tail), exact ScalarE tanh, filler matmuls
    to keep the PE clock warm.
  * Projections (out = W_out.T @ [x_own; h_final], all bf16) are scheduled
    in early tail steps where the PE is latency-bound, chunk 0 at the end.
"""

import math
import os
import sys

for _p in (
    "/opt/trn_rl_repo",
    "/root/.axon_site",
    "/root/.axon_site/_ro/trn_rl_repo",
    "/root/.axon_site/_ro/pypackages",
):
    if os.path.isdir(_p) and _p not in sys.path:
        sys.path.append(_p)

import numpy as np

import concourse.bass as bass
import concourse.tile as tile
from concourse import bacc, mybir
from concourse.bass_utils import run_bass_kernel_spmd

NCORES = 8
F, H, OUT = 128, 256, 256
CH = 512  # chunk width (matmul free dim; one fp32 PSUM bank)

F32 = mybir.dt.float32
BF16 = mybir.dt.bfloat16
FP8 = mybir.dt.float8e4
DRM = mybir.MatmulPerfMode.DoubleRow

_SIG = mybir.ActivationFunctionType.Sigmoid
_TANH = mybir.ActivationFunctionType.Tanh

# Gate layout (after host reorder): blocks [i0 i1 f0 f1 o0 o1 g0 g1].
# Original reference order is [i, f, g, o]; host permutes rows to [i, f, o, g].

# ---------------------------------------------------------- custom DVE ops
# 2-pass deg-4 odd polynomial tanh: tanh(t) ~ t*P(t^2), t = clamp(c, +-R).
# Fit on [0, 2.75]; |c| stays < 2.5 for this model (max abs err 3.6e-3).
R_TANH = 2.75
PC = [0.9829917503288796, -0.2693295191488006, 0.05789263579717002,
      -0.006677166060228195, 0.0003035239700996965]

from concourse.dve_spec import (  # noqa: E402
    C0, C1, C2, C3, One, Spec, Src0, Src1, Zero, _spill_c3_to_src1, lower,
    maxx, minn, sq,
)
from concourse import dve_ops  # noqa: E402
from concourse.dve_uop import DveOpSpec  # noqa: E402


def _register_dve(name, spec, subdim=False):
    """Idempotently register a custom DVE op at import time."""
    if name in dve_ops._SUB_OPCODE_FOR_NAME:
        return next(op for op in dve_ops.OPS if op.name == name)
    opcode = dve_ops._CUSTOM_DVE_ROW_BASE + len(dve_ops.OPS)
    assert opcode < 0x20
    shas = {}
    for ver in ("v3", "v4"):
        s = DveOpSpec(name=name, opcode=opcode, uops=lower(spec, ver=ver),
                      rd1_en=dve_ops.has_src1(spec))
        shas[ver] = s.sha(ver)
    op = dve_ops.DveOp(name, spec, subdim=subdim, uops_sha=shas)
    dve_ops.OPS.append(op)
    dve_ops.CUSTOM_DVE_SPECS[name] = spec
    dve_ops._SUB_OPCODE_FOR_NAME[name] = opcode
    return op


def _p1_body():
    u = minn(sq(Src0), C0)  # C0 = R^2; u-clamp (|c| stays well inside R)
    return _spill_c3_to_src1((u * C1 + C2) * u + C3)


def _p1_ref(in0, in1, c0, c1, c2):
    u = np.minimum(np.square(np.asarray(in0, np.float32)), c0)
    return (u * c1 + c2) * u + in1


def _p2_body():
    # STT struct (2-free-dim in1) has no imm2 slot -> only C0/C1 here.  u is
    # unclamped; instead the output clamps to [-1, 1], which also makes the
    # downstream bf16->fp8 h write overflow-proof for any c.
    u = sq(Src1)
    v = ((Src0 * u + C0) * u + C1) * Src1
    return minn(maxx(v, Zero - One), One)


def _p2_ref(in0, in1, c0, c1, c2):
    c = np.asarray(in1, np.float32)
    u = np.square(c)
    return np.clip(((in0 * u + c0) * u + c1) * c, -1.0, 1.0)


TANH_P1 = _register_dve("TANH_P1", Spec(body=_p1_body(), reference=_p1_ref))
TANH_P2 = _register_dve("TANH_P2", Spec(body=_p2_body(), reference=_p2_ref))


# ---------------------------------------------------------------- host side

def _preprocess(input_matrix, adjacency):
    """Partition nodes, build shared schedule + packed per-core inputs."""
    N = input_matrix.shape[0]
    src, trg = adjacency[0], adjacency[1]

    order = np.argsort(src, kind="stable")
    trg_s = trg[order]
    counts = np.bincount(src, minlength=N).astype(np.int64)
    offsets = np.zeros(N + 1, np.int64)
    np.cumsum(counts, out=offsets[1:])

    rank_order = np.argsort(-counts, kind="stable")
    core_nodes = [rank_order[c::NCORES] for c in range(NCORES)]
    deg_c = [counts[cn] for cn in core_nodes]

    T = int(counts.max())
    cnt = np.zeros((NCORES, T + 1), np.int64)
    for c in range(NCORES):
        h = np.bincount(deg_c[c], minlength=T + 1)
        cs = np.cumsum(h)
        cnt[c, :] = len(deg_c[c]) - cs[: T + 1]
    D = np.max(cnt[:, :-1] - cnt[:, 1:], axis=0)  # D[d-1] for d=1..T
    # every schedule level rounded up to a multiple of 4 (f32r legacy; also
    # keeps the DoubleRow pair strides 16B-aligned for narrow widths)
    M = np.zeros(T + 1, np.int64)
    for t in range(T - 1, -1, -1):
        M[t] = -(-(M[t + 1] + D[t]) // 4) * 4

    ALL_COL = int(M[0])
    col_node = []
    deg0 = []
    for c in range(NCORES):
        cn = np.full(ALL_COL, -1, np.int64)
        for d in range(T, 0, -1):
            s0 = int(cnt[c, d])
            k = int(cnt[c, d - 1]) - s0
            if k:
                cn[int(M[d]) : int(M[d]) + k] = core_nodes[c][s0 : s0 + k]
        deg0.append(core_nodes[c][deg_c[c] == 0])  # handled on host
        col_node.append(cn)

    Mt = M[:-1]
    off = np.zeros(T + 1, np.int64)
    np.cumsum(Mt, out=off[1:])
    S = int(off[T])

    TSW = next((t for t in range(1, T) if int(Mt[t]) <= CH), T)
    S_wide = int(off[TSW])
    S_tail = S - S_wide

    e4 = mybir.dt.np(FP8)
    bf = mybir.dt.np(BF16)
    im32 = np.ascontiguousarray(input_matrix, np.float32)
    x_hi = im32.astype(e4)
    x_lo = (im32 - x_hi.astype(np.float32)).astype(e4)

    xseq8 = []   # [128, 2, S_wide] fp8 per core
    xseqb = []   # [128, S_tail] bf16 per core
    xown = []    # [128, AC] bf16 per core
    for c in range(NCORES):
        cn = col_node[c]
        xs8 = np.zeros((S_wide, 2, F), e4)
        xs8[:, 1, 0] = 1.0  # bias channel rides slot1 row0
        xsb = np.zeros((S_tail, F), np.float32)
        for t in range(T):
            m = int(Mt[t])
            colnodes = cn[:m]
            valid = colnodes >= 0
            vnodes = colnodes[valid]
            nbr = trg_s[offsets[vnodes] + t]
            if t < TSW:
                o = int(off[t])
                blk = xs8[o : o + m]
                blk[valid, 0, :] = x_hi[nbr]
                lo = x_lo[nbr].copy()
                lo[:, 0] = 0.0  # row0 of slot1 is the bias channel
                tmp = blk[valid]
                tmp[:, 1, 1:] = lo[:, 1:]
                blk[valid] = tmp
            else:
                o = int(off[t] - S_wide)
                xsb[o : o + m][valid] = im32[nbr]
        xseq8.append(np.ascontiguousarray(xs8.transpose(2, 1, 0)))
        xseqb.append(np.ascontiguousarray(xsb.T.astype(bf)))
        xo = np.zeros((ALL_COL, F), np.float32)
        valid = cn >= 0
        xo[valid] = im32[cn[valid]]
        xown.append(np.ascontiguousarray(xo.T.astype(bf)))

    return dict(T=T, M=Mt, off=off, S=S, AC=ALL_COL, TSW=TSW,
                xseq8=xseq8, xseqb=xseqb, xown=xown,
                col_node=col_node, deg0=deg0)


def _gate_perm():
    """Row permutation [i, f, g, o] -> [i, f, o, g] on the 4H axis."""
    idx = np.arange(4 * H)
    return np.concatenate([idx[0:2*H], idx[3*H:4*H], idx[2*H:3*H]])


def _make_in_maps(pp, W_ih, W_hh, b_ih, b_hh, W_out):
    e4 = mybir.dt.np(FP8)
    bf = mybir.dt.np(BF16)
    perm = _gate_perm()
    Wi = np.ascontiguousarray(W_ih[perm]).astype(np.float32)   # [4H, F]
    Wh = np.ascontiguousarray(W_hh[perm]).astype(np.float32)   # [4H, H]
    bc = (b_ih + b_hh).astype(np.float32)[perm]                # [4H]

    # fp8 DoubleRow weights.  wih8[k, 0, u] = e4(Wi[u, k]); slot1 row0 = bias,
    # slot1 rows 1.. repeat e4(Wi) so slot1 (x residual) reuses the weight.
    Wi8 = Wi.T.astype(e4)                    # [F, 4H]
    wih8 = np.zeros((F, 2, 4 * H), e4)
    wih8[:, 0, :] = Wi8
    wih8[1:, 1, :] = Wi8[1:]
    wih8[0, 1, :] = bc.astype(e4)
    whh8 = np.zeros((128, 2, 4 * H), e4)
    whh8[:, 0, :] = Wh.T[0:128].astype(e4)
    whh8[:, 1, :] = Wh.T[128:256].astype(e4)

    # bf16 tail weights [3, 128, 4H]: x, hh0, hh1
    wlb = np.stack([
        np.ascontiguousarray(Wi.T),
        np.ascontiguousarray(Wh.T[:128]),
        np.ascontiguousarray(Wh.T[128:]),
    ]).astype(bf)
    wo = np.stack([W_out[0:128], W_out[128:256], W_out[256:384]]).astype(bf)
    bcm = np.ascontiguousarray(bc.reshape(8, 128).T)           # [128, 8]

    bct8 = bcm.T.astype(bf)                                    # [8, 128]
    be8 = np.zeros((8, 8, 64), np.float32)
    be8[np.arange(8), np.arange(8), :] = 1.0
    be4 = np.zeros((4, 4, 128), np.float32)
    be4[np.arange(4), np.arange(4), :] = 1.0

    maps = []
    for c in range(NCORES):
        maps.append({
            "xseq8": pp["xseq8"][c],
            "xseqb": pp["xseqb"][c],
            "xown": pp["xown"][c],
            "wih8": wih8, "whh8": whh8, "wlb": wlb, "wo": wo,
            "bc": bcm, "bct8": bct8,
            "be8": be8.astype(bf), "be4": be4.astype(bf),
        })
    return maps


# ------------------------------------------------------------- bass program

def build_program(T, Mt, off, S, AC, TSW):
    nc = bacc.Bacc("TRN2", target_bir_lowering=False, debug=False,
                   enable_asserts=False)

    S_wide = int(off[TSW])
    S_tail = int(off[T]) - S_wide

    xseq8_d = nc.declare_dram_parameter("xseq8", [128, 2, S_wide], FP8,
                                        isOutput=False)
    xseqb_d = nc.declare_dram_parameter("xseqb", [128, max(S_tail, 1)], BF16,
                                        isOutput=False)
    xown_d = nc.declare_dram_parameter("xown", [128, AC], BF16, isOutput=False)
    wih8_d = nc.declare_dram_parameter("wih8", [128, 2, 1024], FP8,
                                       isOutput=False)
    whh8_d = nc.declare_dram_parameter("whh8", [128, 2, 1024], FP8,
                                       isOutput=False)
    wlb_d = nc.declare_dram_parameter("wlb", [3, 128, 1024], BF16,
                                      isOutput=False)
    wo_d = nc.declare_dram_parameter("wo", [3, 128, 256], BF16, isOutput=False)
    bc_d = nc.declare_dram_parameter("bc", [128, 8], F32, isOutput=False)
    bct8_d = nc.declare_dram_parameter("bct8", [8, 128], BF16, isOutput=False)
    be8_d = nc.declare_dram_parameter("be8", [8, 8, 64], BF16, isOutput=False)
    be4_d = nc.declare_dram_parameter("be4", [4, 4, 128], BF16, isOutput=False)
    out_d = nc.declare_dram_parameter("out", [2, 128, AC], F32, isOutput=True)

    NCH = math.ceil(AC / CH)
    # chunk j>=1 finishes at TSW-1 (last step with M_t > CH); its projection
    # runs in an early tail step where the PE is latency-bound.  chunk 0 last.
    proj_at = [T - 1] + [min(TSW + (j - 1), T - 2) for j in range(1, NCH)]

    with tile.TileContext(nc) as tc:
        with (
            tc.tile_pool(name="const", bufs=1) as constp,
            tc.tile_pool(name="state", bufs=1) as statep,
            tc.tile_pool(name="xin", bufs=8) as xinp,
            tc.tile_pool(name="gates", bufs=3) as gatep,
            tc.tile_pool(name="tmp", bufs=4) as tmpp,
            tc.tile_pool(name="ppool", bufs=3) as ppool,
            tc.tile_pool(name="psumA", bufs=1, space="PSUM") as psumA,
            tc.tile_pool(name="psumB", bufs=1, space="PSUM") as psumB,
            tc.tile_pool(name="outs", bufs=3) as outsp,
        ):
            # -------- weights / constants (gpsimd DMA queue; sync queue is
            # reserved for the x stream so step 0 isn't stuck behind weights)
            wih8 = constp.tile([128, 2, 1024], FP8, tag="wih8")
            bias = constp.tile([128, 8], F32, tag="bias")
            scr = constp.tile([128, 1], F32, tag="scr")
            pc2 = constp.tile([128, 1], F32, tag="pc2")
            nc.gpsimd.dma_start(wih8[:], wih8_d[:])
            nc.gpsimd.dma_start(bias[:], bc_d[:])
            # dummy 1-elem sigmoid pulls the ACT table load into startup
            nc.scalar.activation(scr[:, 0:1], bias[:, 0:1], _SIG)
            nc.vector.memset(pc2[:], PC[2])
            whh8 = constp.tile([128, 2, 1024], FP8, tag="whh8")
            nc.gpsimd.dma_start(whh8[:], whh8_d[:])
            w_x_b = constp.tile([128, 1024], BF16, tag="wxb")
            w_h0_b = constp.tile([128, 1024], BF16, tag="wh0b")
            w_h1_b = constp.tile([128, 1024], BF16, tag="wh1b")
            nc.gpsimd.dma_start(w_x_b[:], wlb_d[0])
            nc.gpsimd.dma_start(w_h0_b[:], wlb_d[1])
            nc.gpsimd.dma_start(w_h1_b[:], wlb_d[2])
            w_o = []
            for k in range(3):
                t_ = constp.tile([128, 256], BF16, tag=f"wo{k}")
                nc.gpsimd.dma_start(t_[:], wo_d[k])
                w_o.append(t_)
            h_b = constp.tile([128, 2, CH], BF16, tag="hb")
            bct8 = constp.tile([8, 128], BF16, tag="bct8")
            bct4b = constp.tile([4, 128], BF16, tag="bct4b")
            be8 = constp.tile([8, 8, 64], BF16, tag="be8")
            be4 = constp.tile([4, 4, 128], BF16, tag="be4")
            nc.gpsimd.dma_start(bct8[:], bct8_d[:])
            nc.gpsimd.dma_start(bct4b[:], bct8_d[4:8])
            nc.gpsimd.dma_start(be8[:], be8_d[:])
            nc.gpsimd.dma_start(be4[:], be4_d[:])

            # -------- state: h fp8 pairs (DoubleRow rhs), c bf16, final-h bf16
            h8_t, c_t, hf_t = [], [], []
            for j in range(NCH):
                h8 = statep.tile([128, 2, CH], FP8, tag=f"h8{j}")
                ct = statep.tile([128, 2, CH], BF16, tag=f"c{j}")
                hf = statep.tile([128, 2, CH], BF16, tag=f"hf{j}")
                h8_t.append(h8)
                c_t.append(ct)
                hf_t.append(hf)

            def wide_step(t):
                m = int(Mt[t])
                o_t = int(off[t])
                m_next = int(Mt[t + 1]) if t + 1 < T else 0
                for j0 in range(0, m, CH):
                    j = j0 // CH
                    w = min(CH, m - j0)
                    xt = xinp.tile([128, 2, CH], FP8, tag="x")
                    nc.sync.dma_start(xt[:, :, :w],
                                      xseq8_d[:, :, o_t + j0 : o_t + j0 + w])
                    psA = psumA.tile([128, 4, CH], F32, tag="psA")
                    psB = psumB.tile([128, 4, CH], F32, tag="psB")

                    def gates_mm(blk, ps, k):
                        sl = slice(blk * 128, (blk + 1) * 128)
                        nc.tensor.matmul(ps[:, k, :w], wih8[:, :, sl],
                                         xt[:, :, :w], start=True,
                                         stop=(t == 0), perf_mode=DRM)
                        if t > 0:
                            nc.tensor.matmul(ps[:, k, :w], whh8[:, :, sl],
                                             h8_t[j][:, :, :w], start=False,
                                             stop=True, perf_mode=DRM)

                    for k in range(4):
                        gates_mm(k, psA, k)       # blocks i0 i1 f0 f1
                    for k in range(4):
                        gates_mm(4 + k, psB, k)   # blocks o0 o1 g0 g1

                    G = gatep.tile([128, 8, CH], BF16, tag="G")
                    nc.scalar.activation(G[:, 0:4, :w], psA[:, :, :w], _SIG)
                    nc.scalar.activation(G[:, 4:6, :w], psB[:, 0:2, :w], _SIG)
                    nc.scalar.activation(G[:, 6:8, :w], psB[:, 2:4, :w], _TANH)

                    cv = c_t[j][:, :, :w]
                    t1 = tmpp.tile([128, 2, CH], BF16, tag="t1")
                    if t == 0:
                        nc.gpsimd.tensor_mul(cv, G[:, 0:2, :w], G[:, 6:8, :w])
                    else:
                        nc.gpsimd.tensor_mul(t1[:, :, :w], G[:, 0:2, :w],
                                             G[:, 6:8, :w])
                        cf = tmpp.tile([128, 2, CH], BF16, tag="cf")
                        nc.vector.tensor_mul(cf[:, :, :w], cv, G[:, 2:4, :w])
                        nc.vector.tensor_add(cv, cf[:, :, :w], t1[:, :, :w])
                    pp_ = ppool.tile([128, 2, CH], F32, tag="p")
                    th = ppool.tile([128, 2, CH], BF16, tag="th")
                    nc.vector._custom_dve(TANH_P1, out=pp_[:, :, :w], in0=cv,
                                          in1=pc2[:], s0=R_TANH * R_TANH, s1=PC[4],
                                          imm2=PC[3])
                    nc.vector._custom_dve(TANH_P2, out=th[:, :, :w],
                                          in0=pp_[:, :, :w], in1=cv,
                                          s0=PC[1], s1=PC[0])
                    nc.vector.tensor_mul(h8_t[j][:, :, :w], G[:, 4:6, :w],
                                         th[:, :, :w])
                    # bf16 final-h for columns whose lifetime ends at t
                    lo = max(m_next, j0)
                    hi = min(m, j0 + CH)
                    if lo < hi:
                        ll, hh = lo - j0, hi - j0
                        nc.vector.tensor_mul(hf_t[j][:, :, ll:hh],
                                             G[:, 4:6, ll:hh], th[:, :, ll:hh])

            # -------------------- tail (bf16, baseline machinery) --------
            def h_rhs(half, w):
                return h_b[:, half, :w]

            def tail_step(t):
                m = int(Mt[t])
                o_t = int(off[t])
                w = m
                xt = xinp.tile([128, CH], BF16, tag="xb")
                ob = o_t - S_wide
                nc.sync.dma_start(xt[:, :w], xseqb_d[:, ob : ob + w])

                psA = psumA.tile([128, 4, CH], F32, tag="psA")
                psB = psumB.tile([128, 4, CH], F32, tag="psB")
                G = gatep.tile([128, 8, CH], BF16, tag="G")

                if w <= 128:
                    # Deep tail: pack 4 or 8 gate blocks in 1-2 banks. Bias
                    # lands first via one delta-pattern matmul per bank.
                    nb = 1 if w <= 64 else 2
                    bpb = 8 // nb
                    be = be8 if nb == 1 else be4
                    psv = []
                    for bnk in range(nb):
                        ps = psA[:, bnk, :]
                        pv = ps.rearrange("p (k c) -> p k c", k=bpb)
                        psv.append(pv)
                        blt = bct8[0:bpb, :] if bnk == 0 else bct4b[:]
                        nc.tensor.matmul(ps[:, :], blt, be[:, :, :],
                                         start=True, stop=False,
                                         skip_group_check=True)
                        for k in range(bpb):
                            mi = bnk * bpb + k
                            sl = slice(mi * 128, (mi + 1) * 128)
                            nc.tensor.matmul(pv[:, k, :w],
                                             w_x_b[:, sl], xt[:, :w],
                                             start=False, stop=False,
                                             skip_group_check=True)
                            nc.tensor.matmul(pv[:, k, :w], w_h0_b[:, sl],
                                             h_rhs(0, w), start=False,
                                             stop=False,
                                             skip_group_check=True)
                            nc.tensor.matmul(pv[:, k, :w], w_h1_b[:, sl],
                                             h_rhs(1, w), start=False,
                                             stop=(k == bpb - 1),
                                             skip_group_check=True)
                    if nb == 1:
                        pv = psv[0]
                        nc.scalar.activation(G[:, 0:6, :w], pv[:, 0:6, :w], _SIG)
                        nc.scalar.activation(G[:, 6:8, :w], pv[:, 6:8, :w], _TANH)
                    else:
                        nc.scalar.activation(G[:, 0:4, :w], psv[0][:, :, :w], _SIG)
                        nc.scalar.activation(G[:, 4:6, :w], psv[1][:, 0:2, :w], _SIG)
                        nc.scalar.activation(G[:, 6:8, :w], psv[1][:, 2:4, :w], _TANH)
                else:
                    # mid tail: per-block bf16 matmuls, bias via ACT operand
                    for mi in (0, 1, 6, 7, 2, 3, 4, 5):  # i, g first (DVE)
                        ps = (psA[:, mi, :w] if mi < 4 else psB[:, mi - 4, :w])
                        sl = slice(mi * 128, (mi + 1) * 128)
                        nc.tensor.matmul(ps, w_x_b[:, sl], xt[:, :w],
                                         start=True, stop=False)
                        nc.tensor.matmul(ps, w_h0_b[:, sl],
                                         h_rhs(0, w), start=False, stop=False)
                        nc.tensor.matmul(ps, w_h1_b[:, sl],
                                         h_rhs(1, w), start=False, stop=True)
                        nc.scalar.activation(
                            G[:, mi, :w], ps,
                            _TANH if mi >= 6 else _SIG,
                            bias=bias[:, mi : mi + 1])

                cv = c_t[0][:, :, :w]
                th = ppool.tile([128, 2, CH], BF16, tag="th")
                t1 = tmpp.tile([128, 2, CH], BF16, tag="t1")
                cf = tmpp.tile([128, 2, CH], BF16, tag="cf")
                nc.vector.tensor_mul(t1[:, :, :w], G[:, 0:2, :w], G[:, 6:8, :w])
                nc.vector.tensor_mul(cf[:, :, :w], cv, G[:, 2:4, :w])
                nc.vector.tensor_add(cv, cf[:, :, :w], t1[:, :, :w])
                nc.scalar.activation(th[:, :, :w], cv, _TANH)
                nc.vector.tensor_mul(h_b[:, :, :w], G[:, 4:6, :w],
                                     th[:, :, :w])

                # latency-bound deep tail: filler matmuls keep HAM busy
                if w <= 300:
                    for _d in range(6):
                        psd = psumB.tile([128, 4, CH], F32, tag="psB")
                        nc.tensor.matmul(psd[:, 3, :CH], w_x_b[:, 0:128],
                                         w_x_b[:, 0:CH],
                                         start=True, stop=True,
                                         skip_group_check=True)

            def projection(j, t):
                j0 = j * CH
                w = min(CH, AC - j0)
                xo = xinp.tile([128, CH], BF16, tag="xo")
                nc.sync.dma_start(xo[:, :w], xown_d[:, j0 : j0 + w])
                if j == 0:
                    ph0, ph1 = h_b[:, 0, :w], h_b[:, 1, :w]
                else:
                    ph0 = hf_t[j][:, 0, :w]
                    ph1 = hf_t[j][:, 1, :w]
                for mb in range(2):
                    psP = psumB.tile([128, 4, CH], F32, tag="psB")
                    ps = psP[:, mb, :]
                    sl = slice(mb * 128, (mb + 1) * 128)
                    nc.tensor.matmul(ps[:, :w], w_o[0][:, sl], xo[:, :w],
                                     start=True, stop=False)
                    nc.tensor.matmul(ps[:, :w], w_o[1][:, sl], ph0,
                                     start=False, stop=False)
                    nc.tensor.matmul(ps[:, :w], w_o[2][:, sl], ph1,
                                     start=False, stop=True)
                    ot = outsp.tile([128, CH], F32, tag="ot")
                    nc.vector.tensor_copy(ot[:, :w], ps[:, :w])
                    nc.sync.dma_start(out_d[mb, :, j0 : j0 + w], ot[:, :w])

            # ------------------------------ main schedule ----------------
            for t in range(T):
                if t < TSW:
                    wide_step(t)
                    if t == TSW - 1:
                        # handoff chunk 0 state to the bf16 tail: active
                        # columns from fp8 h, finished columns from hf
                        wc = min(CH, AC)
                        mA = int(Mt[TSW]) if TSW < T else 0
                        if mA > 0:
                            nc.vector.tensor_copy(h_b[:, :, :mA],
                                                  h8_t[0][:, :, :mA])
                        if wc > mA:
                            nc.vector.tensor_copy(h_b[:, :, mA:wc],
                                                  hf_t[0][:, :, mA:wc])
                else:
                    tail_step(t)
                for j in range(NCH):
                    if proj_at[j] == t:
                        projection(j, t)

    nc.compile()
    return nc


# ------------------------------------------------------------------ kernel

def run(inputs, trace=False, mm_dt=None):
    """Full pipeline; returns (output [N, OUT], BassKernelResults, pp)."""
    input_matrix = np.asarray(inputs["input_matrix"], np.float32)
    adjacency = np.asarray(inputs["adjacency"])
    W_ih = np.asarray(inputs["W_ih"], np.float32)
    W_hh = np.asarray(inputs["W_hh"], np.float32)
    b_ih = np.asarray(inputs["b_ih"], np.float32)
    b_hh = np.asarray(inputs["b_hh"], np.float32)
    W_out = np.asarray(inputs["W_out"], np.float32)

    pp = _preprocess(input_matrix, adjacency)
    nc = build_program(pp["T"], pp["M"], pp["off"], pp["S"], pp["AC"],
                       pp["TSW"])
    in_maps = _make_in_maps(pp, W_ih, W_hh, b_ih, b_hh, W_out)
    res = run_bass_kernel_spmd(nc, in_maps, list(range(NCORES)), trace=trace)

    N = input_matrix.shape[0]
    out = np.zeros((N, OUT), np.float32)
    for c in range(NCORES):
        oc = np.asarray(res.results[c]["out"]).reshape(OUT, pp["AC"])
        cn = pp["col_node"][c]
        valid = cn >= 0
        out[cn[valid]] = oc[:, valid].T
        if len(pp["deg0"][c]):
            z = pp["deg0"][c]
            out[z] = input_matrix[z] @ W_out[:F]  # h = 0 for degree-0 nodes
    return out, res, pp


def kernel(**inputs) -> np.ndarray:
    out, _, _ = run(inputs, trace=False)
    return out
